# revision 13
# baseline (speedup 1.0000x reference)
"""LRU (Linear Recurrent Unit) single-step forward on 8 Trainium2 NeuronCores.

Math: with seq-len 1 the whole LRU step collapses algebraically to one GEMM:
    y[b,:] = W @ u[b] + bias
where
    W    = 2*C_re@diag(g)@B_re - 2*C_im@diag(g)@B_im + D          [DOUT, DIN]
    bias = 2*(C_re@(lam_re*x_re - lam_im*x_im)
              - C_im@(lam_re*x_im + lam_im*x_re))                  [DOUT]
    g = exp(gamma_log), lam = exp(-exp(nu_log)) * exp(i*exp(theta_log)).

The parameter fold (W, bias) is input-data independent (depends only on the
model parameters / initial state), computed once on host in float64.  The
batch GEMM (99% of FLOPs and bytes) runs on the 8 NeuronCores, data-parallel
over the batch: each core computes y_shard^T = W @ u_shard^T (+bias).

Device kernel layout (per core):
  ub [4, 8, 128, 512]  u^T blocked (bt, ib, i-part, b)  - contiguous tiles
  wt [1024, 1024]      W^T (i, j) - row blocks of 128 are contiguous tiles
  bias [128, 8]        bias[jb*128+p] at [p, jb]
  yb [4, 8, 128, 512]  y^T blocked (bt, jb, j-part, b)  - contiguous tiles
PE: out[jb-block, b-tile] += wt_tile[ib]^T(128x128) @ u_tile[ib](128x512),
float32r (full-rate fp32 mode).  Batch-tile 0 runs K-outer so the PE can
start as soon as the first (W, u) block pair lands; later tiles run
jb-outer so PSUM drains spread out.  ACT and DVE alternate on the
PSUM->SBUF bias-add drain.
"""

import numpy as np

BATCH, DIN, DSTATE, DOUT = 16384, 1024, 2048, 1024
N_CORES = 8
B_SHARD = BATCH // N_CORES  # 2048 rows per core
P = 128                     # SBUF partitions
NB = 512                    # batch tile (moving free dim, max 512)
I_BLOCKS = DIN // P         # 8 contraction blocks
J_BLOCKS = DOUT // P        # 8 output-row blocks
B_TILES = B_SHARD // NB     # 4 batch tiles per core
N_WARM = 14                 # PE warm-up matmuls (HAM clock-gate release)

_CACHE = {}


def _build_nc():
    import concourse.mybir as mybir
    import concourse.tile as tile
    from concourse import bacc
    from concourse._compat import get_trn_type

    nc = bacc.Bacc(get_trn_type() or "TRN2", target_bir_lowering=False)
    f32 = mybir.dt.float32
    f16 = mybir.dt.float16
    f32r = mybir.dt.float32r  # full-rate fp32 matmul mode on TRN2
    act_id = mybir.ActivationFunctionType.Identity

    ub = nc.declare_dram_parameter("ub", [B_TILES, I_BLOCKS, P, NB], f32r,
                                   isOutput=False)
    # W^T ships as fp16 (halves the critical-path prefill DMA; W rounding
    # ~5e-4 relative, far inside tolerance) and is upcast on-device.
    wt = nc.declare_dram_parameter("wt", [DIN, DOUT], f16, isOutput=False)
    bias = nc.declare_dram_parameter("bias", [P, J_BLOCKS], f32, isOutput=False)
    yb = nc.declare_dram_parameter("yb", [B_TILES, J_BLOCKS, P, NB], f32,
                                   isOutput=True)

    with tile.TileContext(nc) as tc:
        with (
            tc.tile_pool(name="consts", bufs=1) as consts,
            tc.tile_pool(name="upool", bufs=3) as upool,
            tc.tile_pool(name="opool", bufs=6) as opool,
            tc.tile_pool(name="psum", bufs=8, space="PSUM") as psum,
        ):
            bias_t = consts.tile([P, J_BLOCKS], f32, tag="bias")
            nc.sync.dma_start(out=bias_t[:], in_=bias[:])

            # PE warm-up: the HAM clock gate keeps the PE at 1.2 GHz until it
            # has been busy a full ~3.4us activity window.  Run junk matmuls
            # (memset tiles, no DMA deps) during the DMA prefill so the real
            # stream runs at 2.4 GHz.
            warm_w = consts.tile([P, P], f32, tag="warm_w")
            warm_u = consts.tile([P, NB], f32, tag="warm_u")
            nc.gpsimd.memset(warm_w[:], 0.0)
            nc.gpsimd.memset(warm_u[:], 0.0)
            warm_p = psum.tile([P, NB], f32, tag="pt", name="warm_p")
            for _ in range(N_WARM):
                nc.tensor.matmul(warm_p[:], warm_w[:].bitcast(f32r),
                                 warm_u[:].bitcast(f32r),
                                 start=True, stop=True)

            # Interleave W-block and first-batch-tile u loads: matmuls of
            # contraction block ib need only pair ib, so the PE starts after
            # ~0.8 MiB instead of ~6 MiB.
            w_tiles = []
            u_tiles0 = []
            for ib in range(I_BLOCKS):
                w16 = upool.tile([P, DOUT], f16, tag="w16", name=f"w16_{ib}")
                nc.sync.dma_start(out=w16[:], in_=wt[ib * P:(ib + 1) * P, :])
                w_t = consts.tile([P, DOUT], f32r, tag=f"w{ib}", name=f"w{ib}")
                nc.vector.tensor_copy(w_t[:], w16[:])  # fp16 -> fp32r upcast
                w_tiles.append(w_t)
                u_t = upool.tile([P, NB], f32r, tag=f"u{ib}", name=f"u{ib}_0")
                nc.sync.dma_start(out=u_t[:], in_=ub[0, ib])
                u_tiles0.append(u_t)

            def drain(jb, bt, pt):
                """PSUM -> SBUF with per-partition bias add, then store."""
                ot = opool.tile([P, NB], f32, tag="ot", name=f"ot_{bt}_{jb}")
                if jb % 2 == 0:
                    nc.scalar.activation(ot[:], pt[:], act_id,
                                         bias=bias_t[:, jb:jb + 1])
                else:
                    nc.vector.tensor_scalar_add(ot[:], pt[:],
                                                bias_t[:, jb:jb + 1])
                nc.sync.dma_start(out=yb[bt, jb], in_=ot[:])

            for bt in range(B_TILES):
                if bt == 0:
                    u_tiles = u_tiles0
                else:
                    u_tiles = []
                    for ib in range(I_BLOCKS):
                        u_t = upool.tile([P, NB], f32r, tag=f"u{ib}",
                                         name=f"u{ib}_{bt}")
                        nc.sync.dma_start(out=u_t[:], in_=ub[bt, ib])
                        u_tiles.append(u_t)
                if bt == 0:
                    # K-outer: all 8 PSUM groups in flight; each arriving
                    # (W, u) pair unlocks one matmul in every group.
                    pts = [psum.tile([P, NB], f32, tag="pt", name=f"pt_{bt}_{jb}")
                           for jb in range(J_BLOCKS)]
                    for ib in range(I_BLOCKS):
                        for jb in range(J_BLOCKS):
                            nc.tensor.matmul(
                                pts[jb][:],
                                w_tiles[ib][:, jb * P:(jb + 1) * P],
                                u_tiles[ib][:],
                                start=(ib == 0),
                                stop=(ib == I_BLOCKS - 1),
                            )
                    for jb in range(J_BLOCKS):
                        drain(jb, bt, pts[jb])
                else:
                    # jb-outer: drains spread across the batch tile.
                    for jb in range(J_BLOCKS):
                        pt = psum.tile([P, NB], f32, tag="pt",
                                       name=f"pt_{bt}_{jb}")
                        for ib in range(I_BLOCKS):
                            nc.tensor.matmul(
                                pt[:],
                                w_tiles[ib][:, jb * P:(jb + 1) * P],
                                u_tiles[ib][:],
                                start=(ib == 0),
                                stop=(ib == I_BLOCKS - 1),
                            )
                        drain(jb, bt, pt)
    nc.compile()
    return nc


def _fold_params(x_re, x_im, nu_log, theta_log, gamma_log, B_re, B_im, C_re, C_im, D):
    """Fold the LRU parameters into (W^T [DIN, DOUT], bias [DOUT]) in float64."""
    nu = np.asarray(nu_log, np.float64)
    th = np.exp(np.asarray(theta_log, np.float64))
    lam_mod = np.exp(-np.exp(nu))
    lam_re = lam_mod * np.cos(th)
    lam_im = lam_mod * np.sin(th)
    g = np.exp(np.asarray(gamma_log, np.float64))
    C_re64 = np.asarray(C_re, np.float64)
    C_im64 = np.asarray(C_im, np.float64)
    W = (2.0 * ((C_re64 * g) @ np.asarray(B_re, np.float64))
         - 2.0 * ((C_im64 * g) @ np.asarray(B_im, np.float64))
         + np.asarray(D, np.float64))  # [DOUT, DIN]
    xr = np.asarray(x_re, np.float64)
    xi = np.asarray(x_im, np.float64)
    lx_re = lam_re * xr - lam_im * xi
    lx_im = lam_re * xi + lam_im * xr
    bias = 2.0 * (C_re64 @ lx_re - C_im64 @ lx_im)  # [DOUT]
    return W.T.astype(np.float32).copy(), bias.astype(np.float32)


def kernel(u_in, x_re, x_im, nu_log, theta_log, gamma_log, B_re, B_im,
           C_re, C_im, D, _trace=False):
    from concourse.bass_utils import run_bass_kernel_spmd

    wt_host, bias_host = _fold_params(
        x_re, x_im, nu_log, theta_log, gamma_log, B_re, B_im, C_re, C_im, D)
    wt16_host = wt_host.astype(np.float16)
    bias2 = np.ascontiguousarray(bias_host.reshape(J_BLOCKS, P).T)  # [128, 8]

    u2 = np.asarray(u_in, np.float32).reshape(BATCH, DIN)
    core_ids = list(range(N_CORES))
    in_maps = []
    for c in core_ids:
        shard = u2[c * B_SHARD:(c + 1) * B_SHARD]          # [2048, 1024]
        # ub[bt, ib, p, n] = shard[bt*NB + n, ib*P + p]
        ubc = np.ascontiguousarray(
            shard.reshape(B_TILES, NB, I_BLOCKS, P).transpose(0, 2, 3, 1))
        in_maps.append({"ub": ubc, "wt": wt16_host, "bias": bias2})

    if "nc" not in _CACHE:
        _CACHE["nc"] = _build_nc()
    res = run_bass_kernel_spmd(_CACHE["nc"], in_maps, core_ids, trace=_trace)

    y = np.empty((BATCH, DOUT), np.float32)
    for c in core_ids:
        # yb[bt, jb, p, n] = y_shard[bt*NB + n, jb*P + p]
        ybc = res.results[c]["yb"]
        y[c * B_SHARD:(c + 1) * B_SHARD] = (
            ybc.transpose(0, 3, 1, 2).reshape(B_SHARD, DOUT))
    out = y.reshape(BATCH, 1, DOUT)
    if _trace:
        return out, res
    return out
